# revision 1
# baseline (speedup 1.0000x reference)
"""Bahdanau attention fused kernel for Trainium2, 8-core data-parallel.

Reference computation (per batch b of 32, H=1024, S=2048):
    enc_score = encoder_out @ We + be                    [B, S, H]
    dec_score = dec @ Wd + bd                            [B, 1, H]
    score     = tanh(enc_score + dec_score)              [B, S, H]
    ls        = score @ Ws + bs                          [B, S, 1]
    w         = softmax(ls, axis=S)
    out       = sum_s w[b,s] * encoder_out[b,s,:]        [B, H]

Sharding: batch 32 -> 4 per core across 8 cores; weights replicated.
The tiny dec-score GEMM (67 MFLOP of 137 GFLOP, 0.05%) is folded into the
host-side bias preparation: bias[b] = be + bd + dec[b] @ Wd. bs is dropped
(softmax is shift-invariant). No max-subtraction in softmax: |ls| <= 16.

Per-core device layout (everything h-partitioned, prepared host-side):
    xt   [4, 4, 128, 8*512] bf16  xt[b, c, p, k*512+s'] = X[b, c*512+s', k*128+p]
    we   [128, 8*1024]      bf16  we[p, k*1024+n]       = We[k*128+p, n]
    ws   [128, 8]           bf16  ws[p, j]              = Ws[j*128+p, 0]
    bias [128, 32]          f32   bias[p, j*4+b]        = (be+bd+dec[b]@Wd)[j*128+p]
    out: ctx [4, 128, 8]    f32   ctx[b, p, j]          = out[b, j*128+p]

Device schedule per batch b (PE-bound, ~221us roofline/core at bf16):
  - enc_score.T tiles via matmul: We (stationary) x X.T (moving), 8 k-tiles
    accumulated in PSUM -> [128 h_out, 512 s]; issue cadence is the N=512
    streaming limit (~216 ns/matmul)
  - ScalarE evacuates PSUM with fused tanh(psum + bias[b,j]) -> bf16
  - ls.T = sum_j Ws[j-tile].T @ tanh-tile, accumulated in PSUM [1, 512]
  - ScalarE exp (bf16) with fused accum_out denominator (fp32)
  - ONLINE context: per s-chunk, raw exp weights are broadcast to 128
    partitions via a ones-matmul, multiplied against the cached X.T chunk
    (VectorE) and partial-reduced per k-tile (VectorE; ScalarE accum_out
    for each batch's last chunk); the softmax denominator is divided out
    once per batch. The context of chunk c is emitted after the matmuls of
    chunk c+1 so the PE never waits on the softmax chain.
"""

import numpy as np
import ml_dtypes

import concourse.tile as tile
from concourse import bacc, mybir
from concourse.bass_utils import run_bass_kernel_spmd

BF16 = mybir.dt.bfloat16
F32 = mybir.dt.float32
AF = mybir.ActivationFunctionType

N_CORES = 8
H = 1024
S = 2048
B_PER_CORE = 4
S_CHUNK = 512

# test.py can flip this to get a profiled run; the grading path never does.
PROFILE = {"trace": False, "tmpdir": None}


def build_program(b_per_core=B_PER_CORE, s=S, h=H):
    kt = h // 128
    jt = h // 128
    n_sc = s // S_CHUNK
    nc = bacc.Bacc("TRN2", target_bir_lowering=False, debug=False)

    xt_d = nc.dram_tensor(
        "xt", [b_per_core, n_sc, 128, kt * S_CHUNK], BF16, kind="ExternalInput"
    ).ap()
    we_d = nc.dram_tensor("we", [128, kt * h], BF16, kind="ExternalInput").ap()
    ws_d = nc.dram_tensor("ws", [128, jt], BF16, kind="ExternalInput").ap()
    bias_d = nc.dram_tensor(
        "bias", [128, jt * b_per_core], F32, kind="ExternalInput"
    ).ap()
    ctx_d = nc.dram_tensor("ctx", [b_per_core, 128, jt], F32, kind="ExternalOutput").ap()

    with tile.TileContext(nc) as tc:
        with (
            tc.tile_pool(name="consts", bufs=1) as consts,
            tc.tile_pool(name="xtp", bufs=12) as xtp,
            tc.tile_pool(name="scorep", bufs=10) as scorep,
            tc.tile_pool(name="smallp", bufs=2 * n_sc) as smallp,
            tc.tile_pool(name="ebcp", bufs=2 * n_sc) as ebcp,
            tc.tile_pool(name="scrp", bufs=6) as scrp,
            tc.tile_pool(name="trashp", bufs=1) as trashp,
            tc.tile_pool(name="ctxp", bufs=2) as ctxp,
            tc.tile_pool(name="ps_main", bufs=4, space="PSUM") as ps_main,
            tc.tile_pool(name="ps_ls", bufs=2, space="PSUM") as ps_ls,
            tc.tile_pool(name="ps_misc", bufs=2, space="PSUM") as ps_misc,
        ):
            # we goes FIRST on the sync ring, ahead of the xt stream: with
            # the scalar ring nearly empty, the sync ring gets all 16 SDMA
            # engines, so the first-matmul gate (we + xt[0,0]) clears at
            # full HBM bandwidth instead of splitting it with prefetch.
            we_sb = consts.tile([128, kt * h], BF16)
            nc.sync.dma_start(we_sb[:], we_d[:])
            ws_sb = consts.tile([128, jt], BF16)
            nc.scalar.dma_start(ws_sb[:], ws_d[:])
            bias_sb = consts.tile([128, jt * b_per_core], F32)
            nc.scalar.dma_start(bias_sb[:], bias_d[:])
            ones_bf = consts.tile([1, 128], BF16)
            nc.vector.memset(ones_bf[:], 1.0)
            ones_f32 = consts.tile([1, 128], F32)
            nc.vector.memset(ones_f32[:], 1.0)

            def emit_context_chunk(xt_bc, ex, ctx4_b, c, last_chunk, tail=False):
                """Broadcast chunk weights and accumulate context partials.

                The broadcast runs on the otherwise-idle GpSimd engine except
                on the kernel's final chunk, where the PE is idle and the
                ones-matmul + cast path has lower latency.
                """
                ebc = ebcp.tile([128, S_CHUNK], BF16, tag="ebc")
                if tail:
                    bc_ps = ps_misc.tile([128, S_CHUNK], F32, tag="misc")
                    nc.tensor.matmul(
                        bc_ps[:], lhsT=ones_bf[:], rhs=ex[:], start=True, stop=True
                    )
                    nc.vector.tensor_copy(ebc[:], bc_ps[:])
                else:
                    nc.gpsimd.partition_broadcast(ebc[:], ex[:])
                for k in range(kt):
                    scr = scrp.tile([128, S_CHUNK], BF16, tag="scr")
                    nc.vector.tensor_mul(
                        scr[:], xt_bc[:, k * S_CHUNK : (k + 1) * S_CHUNK], ebc[:]
                    )
                    if last_chunk and k % 2 == 0:
                        trash = trashp.tile([128, S_CHUNK], BF16, tag="trash")
                        nc.scalar.activation(
                            trash[:], scr[:], AF.Identity,
                            accum_out=ctx4_b[:, k * n_sc + c : k * n_sc + c + 1],
                        )
                    else:
                        nc.vector.reduce_sum(
                            ctx4_b[:, k * n_sc + c : k * n_sc + c + 1],
                            scr[:],
                            axis=mybir.AxisListType.X,
                        )

            def emit_invd(denom_b):
                """softmax denominator -> broadcast 1/d [128, 1]."""
                dsum = smallp.tile([1, 1], F32, tag="dsum")
                nc.vector.reduce_sum(dsum[:], denom_b[:], axis=mybir.AxisListType.X)
                invd = smallp.tile([1, 1], F32, tag="invd")
                nc.vector.reciprocal(invd[:], dsum[:])
                iv_ps = ps_misc.tile([128, S_CHUNK], F32, tag="misc")
                nc.tensor.matmul(
                    iv_ps[:, 0:1], lhsT=ones_f32[:], rhs=invd[:], start=True, stop=True
                )
                invd_bc = smallp.tile([128, 1], F32, tag="invdbc")
                nc.scalar.copy(invd_bc[:], iv_ps[:, 0:1])
                return invd_bc

            def emit_batch_final(b, ctx4_b, invd_bc):
                """Partial reduction, normalize, store."""
                ctxu = ctxp.tile([128, jt], F32, tag="ctxu")
                for k in range(kt):
                    nc.vector.reduce_sum(
                        ctxu[:, k : k + 1],
                        ctx4_b[:, k * n_sc : (k + 1) * n_sc],
                        axis=mybir.AxisListType.X,
                    )
                ctx_b = ctxp.tile([128, jt], F32, tag="ctx")
                nc.vector.tensor_scalar_mul(ctx_b[:], ctxu[:], invd_bc[:])
                nc.sync.dma_start(ctx_d[b], ctx_b[:])

            pending = []  # deferred (context-chunk | invd | batch-final)
            for b in range(b_per_core):
                xt_tiles = []
                for c in range(n_sc):
                    xt_bc = xtp.tile([128, kt * S_CHUNK], BF16, tag="xt")
                    if b == 0 and c == 0:
                        # split the gate-opening chunk so the first matmul
                        # group starts on the early half
                        half = kt // 2 * S_CHUNK
                        nc.sync.dma_start(xt_bc[:, :half], xt_d[b, c][:, :half])
                        nc.sync.dma_start(xt_bc[:, half:], xt_d[b, c][:, half:])
                    else:
                        nc.sync.dma_start(xt_bc[:], xt_d[b, c])
                    xt_tiles.append(xt_bc)

                denom_b = smallp.tile([1, n_sc], F32, tag="denom")
                ctx4_b = ctxp.tile([128, kt * n_sc], F32, tag="ctx4")
                for c in range(n_sc):
                    ls_ps = ps_ls.tile([1, S_CHUNK], F32, tag="ls")
                    score_tiles = []
                    for j in range(jt):
                        mm_ps = ps_main.tile([128, S_CHUNK], F32, tag="main")
                        for k in range(kt):
                            nc.tensor.matmul(
                                mm_ps[:],
                                lhsT=we_sb[:, k * h + j * 128 : k * h + (j + 1) * 128],
                                rhs=xt_tiles[c][:, k * S_CHUNK : (k + 1) * S_CHUNK],
                                start=(k == 0),
                                stop=(k == kt - 1),
                            )
                        sc = scorep.tile([128, S_CHUNK], BF16, tag="score")
                        nc.scalar.activation(
                            sc[:], mm_ps[:], AF.Tanh,
                            bias=bias_sb[:, j * b_per_core + b : j * b_per_core + b + 1],
                        )
                        score_tiles.append(sc)
                        if j == 0:
                            # deferred work from the previous chunk/batch is
                            # emitted right after the first matmul group, so
                            # its PE ops (weight broadcast) slot in early and
                            # the DVE context work overlaps this chunk's
                            # remaining matmul groups
                            for fn in pending:
                                fn()
                            pending = []
                    for j in range(jt):
                        nc.tensor.matmul(
                            ls_ps[:],
                            lhsT=ws_sb[:, j : j + 1],
                            rhs=score_tiles[j][:],
                            start=(j == 0),
                            stop=(j == jt - 1),
                        )
                    ex = smallp.tile([1, S_CHUNK], BF16, tag="exp")
                    nc.scalar.activation(
                        ex[:], ls_ps[:], AF.Exp, accum_out=denom_b[:, c : c + 1]
                    )

                    last_b = b == b_per_core - 1
                    ctx_fn = (
                        lambda xt_bc=xt_tiles[c], ex=ex, ctx4_b=ctx4_b, c=c,
                        lc=(c == n_sc - 1), tl=(last_b and c == n_sc - 1):
                        emit_context_chunk(xt_bc, ex, ctx4_b, c, lc, tail=tl)
                    )
                    if c < n_sc - 1:
                        pending.append(ctx_fn)
                    elif last_b:
                        # tail of the whole kernel: get 1/d going on the
                        # still-empty DVE queue, then the final context chunk
                        invd_bc = emit_invd(denom_b)
                        ctx_fn()
                        emit_batch_final(b, ctx4_b, invd_bc)
                    else:
                        def batch_tail(ctx_fn=ctx_fn, b=b, ctx4_b=ctx4_b,
                                       denom_b=denom_b):
                            invd_bc = emit_invd(denom_b)
                            ctx_fn()
                            emit_batch_final(b, ctx4_b, invd_bc)
                        pending.append(batch_tail)

    nc.compile()
    return nc


_CACHED = {}


def _get_program(key):
    if key not in _CACHED:
        _CACHED[key] = build_program(*key)
    return _CACHED[key]


def make_in_maps(encoder_out, decoder_hidden_state, We, be, Wd, bd, Ws, bs,
                 b_per_core=B_PER_CORE, s=S, h=H, n_cores=N_CORES):
    kt = h // 128
    jt = h // 128
    n_sc = s // S_CHUNK
    bf = ml_dtypes.bfloat16

    we_a = np.ascontiguousarray(
        We.reshape(kt, 128, h).transpose(1, 0, 2).reshape(128, kt * h)
    ).astype(bf)
    ws_a = np.ascontiguousarray(Ws[:, 0].reshape(jt, 128).T).astype(bf)

    dec = decoder_hidden_state[0]  # [32, h]
    bias_all = (be + bd)[None, :] + dec @ Wd  # [32, h] fp32
    in_maps = []
    for i in range(n_cores):
        b0 = i * b_per_core
        xb = encoder_out[b0 : b0 + b_per_core]  # [b, s, h]
        # [b, c, s', k, p] -> [b, c, p, k, s']
        xt_a = np.ascontiguousarray(
            xb.reshape(b_per_core, n_sc, S_CHUNK, kt, 128).transpose(0, 1, 4, 3, 2)
        ).reshape(b_per_core, n_sc, 128, kt * S_CHUNK).astype(bf)
        bias_a = np.ascontiguousarray(
            bias_all[b0 : b0 + b_per_core].reshape(b_per_core, jt, 128).transpose(2, 1, 0)
        ).reshape(128, jt * b_per_core).astype(np.float32)
        in_maps.append({"xt": xt_a, "we": we_a, "ws": ws_a, "bias": bias_a})
    return in_maps


def kernel(encoder_out, decoder_hidden_state, We, be, Wd, bd, Ws, bs):
    encoder_out = np.asarray(encoder_out, dtype=np.float32)
    decoder_hidden_state = np.asarray(decoder_hidden_state, dtype=np.float32)
    We = np.asarray(We, dtype=np.float32)
    be = np.asarray(be, dtype=np.float32)
    Wd = np.asarray(Wd, dtype=np.float32)
    bd = np.asarray(bd, dtype=np.float32)
    Ws = np.asarray(Ws, dtype=np.float32)
    bs = np.asarray(bs, dtype=np.float32)

    nc = _get_program((B_PER_CORE, S, H))
    in_maps = make_in_maps(
        encoder_out, decoder_hidden_state, We, be, Wd, bd, Ws, bs
    )
    kwargs = {}
    if PROFILE["trace"]:
        kwargs = {"trace": True, "tmpdir": PROFILE["tmpdir"]}
    res = run_bass_kernel_spmd(nc, in_maps, list(range(N_CORES)), **kwargs)
    PROFILE["last_result"] = res

    out = np.empty((N_CORES * B_PER_CORE, H), dtype=np.float32)
    for i in range(N_CORES):
        ctx = res.results[i]["ctx"]  # [b, 128, jt]
        out[i * B_PER_CORE : (i + 1) * B_PER_CORE] = (
            ctx.transpose(0, 2, 1).reshape(B_PER_CORE, H)
        )
    return out



# revision 10
# speedup vs baseline: 1.0079x; 1.0079x over previous
"""Bahdanau attention fused kernel for Trainium2, 8-core data-parallel.

Reference computation (per batch b of 32, H=1024, S=2048):
    enc_score = encoder_out @ We + be                    [B, S, H]
    dec_score = dec @ Wd + bd                            [B, 1, H]
    score     = tanh(enc_score + dec_score)              [B, S, H]
    ls        = score @ Ws + bs                          [B, S, 1]
    w         = softmax(ls, axis=S)
    out       = sum_s w[b,s] * encoder_out[b,s,:]        [B, H]

Sharding: batch 32 -> 4 per core across 8 cores; weights replicated.
The tiny dec-score GEMM is folded into the host-side bias preparation:
bias[b] = be + bd + dec[b] @ Wd. bs is dropped (softmax shift-invariant).
No max-subtraction in softmax: |ls| <= 16.

fp8 version: the big X@We GEMM and the score@Ws projection run in
fp8e4m3 with perf_mode=DoubleRow (2 fp8 weights per PE cell -> one
matmul contracts 256 rows).  The context accumulation keeps a separate
bf16 copy of X and fuses multiply+reduce into single-pass
tensor_tensor_reduce ops on VectorE.

Per-core device layout (h-partitioned, prepared host-side):
    xt8  [4, 4, 128, 8, 512] fp8  xt8[b,c,p,k,s'] = X[b, c*512+s', k*128+p]
    xtb  [4, 4, 128, 8, 512] bf16 same values in bf16 (context path)
    we   [128, 8, 1024]      fp8  we[p,k,n]       = We[k*128+p, n]
    ws   [128, 8, 16]        fp8  ws[p,j,0]       = Ws[j*128+p, 0] (rest 0)
    bias [128, 32]           f32  bias[p, j*4+b]  = (be+bd+dec[b]@Wd)[j*128+p]
    out: ctx [4, 128, 8]     f32  ctx[b,p,j]      = out[b, j*128+p]

Device schedule per (b, c) chunk:
  - 8 j-tiles x 4 DoubleRow matmuls (k-pairs) -> PSUM [128, 512]
  - ScalarE evacuates with fused tanh(psum + bias[b,j]) -> fp8 score,
    written into [128, 2, 512] j-pair tiles
  - ls.T = 4 DoubleRow matmuls (ws j-pairs x score pairs) -> PSUM [1,512]
  - ScalarE exp (bf16) with fused accum_out denominator (fp32)
  - context: exp weights broadcast to 128 partitions on GpSimd, then per
    k-tile one fused tensor_tensor_reduce (VectorE) accumulates
    ctx partials; deferred one chunk so nothing blocks the PE stream.
"""

import numpy as np
import ml_dtypes

import concourse.tile as tile
from concourse import bacc, mybir
from concourse.bass_utils import run_bass_kernel_spmd

FP8 = mybir.dt.float8e4
BF16 = mybir.dt.bfloat16
F32 = mybir.dt.float32
AF = mybir.ActivationFunctionType
ALU = mybir.AluOpType
DR = mybir.MatmulPerfMode.DoubleRow

N_CORES = 8
H = 1024
S = 2048
B_PER_CORE = 4
S_CHUNK = 512

# We/Ws are uniform(-1/32, 1/32) — below e4m3's min normal 2^-6 they
# quantize to subnormals (3.5x the noise).  Scale them up by 64 before
# the fp8 cast and fold 1/64 into the ScalarE activation scale (free).
W_SCALE = 64.0

# Feature flags (HW bring-up bisection)
MAIN_DR = False   # fp8 DoubleRow for the X@We GEMM
LS_DR = False     # fp8 DoubleRow for the score@Ws projection
USE_TTR = False   # fused tensor_tensor_reduce for the context path

# test.py can flip this to get a profiled run; the grading path never does.
PROFILE = {"trace": False, "tmpdir": None}


def build_program(b_per_core=B_PER_CORE, s=S, h=H):
    kt = h // 128
    jt = h // 128
    n_sc = s // S_CHUNK
    nc = bacc.Bacc("TRN2", target_bir_lowering=False, debug=False)

    xt8_d = nc.dram_tensor(
        "xt8", [b_per_core, n_sc, 128, kt, S_CHUNK], FP8, kind="ExternalInput"
    ).ap()
    xtb_d = nc.dram_tensor(
        "xtb", [b_per_core, n_sc, 128, kt, S_CHUNK], BF16, kind="ExternalInput"
    ).ap()
    we_d = nc.dram_tensor("we", [128, kt, h], FP8, kind="ExternalInput").ap()
    ws_d = nc.dram_tensor("ws", [128, jt, 16], FP8, kind="ExternalInput").ap()
    bias_d = nc.dram_tensor(
        "bias", [128, jt * b_per_core], F32, kind="ExternalInput"
    ).ap()
    ctx_d = nc.dram_tensor("ctx", [b_per_core, 128, jt], F32, kind="ExternalOutput").ap()

    with tile.TileContext(nc) as tc:
        with (
            tc.tile_pool(name="consts", bufs=1) as consts,
            tc.tile_pool(name="xt8p", bufs=8) as xt8p,
            tc.tile_pool(name="xtbp", bufs=5) as xtbp,
            tc.tile_pool(name="scorep", bufs=10) as scorep,
            tc.tile_pool(name="smallp", bufs=2 * n_sc) as smallp,
            tc.tile_pool(name="ebcp", bufs=2 * n_sc) as ebcp,
            tc.tile_pool(name="trashp", bufs=2) as trashp,
            tc.tile_pool(name="ctxp", bufs=2) as ctxp,
            tc.tile_pool(name="ps_main", bufs=4, space="PSUM") as ps_main,
            tc.tile_pool(name="ps_ls", bufs=2, space="PSUM") as ps_ls,
            tc.tile_pool(name="ps_misc", bufs=2, space="PSUM") as ps_misc,
        ):
            # we goes FIRST on the sync ring, ahead of the xt stream: with
            # the scalar ring nearly empty, the sync ring gets all 16 SDMA
            # engines, so the first-matmul gate (we + xt[0,0]) clears at
            # full HBM bandwidth instead of splitting it with prefetch.
            we_sb = consts.tile([128, kt, h], FP8)
            nc.sync.dma_start(we_sb[:], we_d[:])
            ws_sb = consts.tile([128, jt, 16], FP8)
            nc.scalar.dma_start(ws_sb[:], ws_d[:])
            bias_sb = consts.tile([128, jt * b_per_core], F32)
            nc.scalar.dma_start(bias_sb[:], bias_d[:])
            ones_bf = consts.tile([1, 128], BF16)
            nc.vector.memset(ones_bf[:], 1.0)
            ones_f32 = consts.tile([1, 128], F32)
            nc.vector.memset(ones_f32[:], 1.0)

            def emit_context_chunk(xtb_bc, ex, ctx4_b, c, tail=False):
                """Broadcast chunk weights and accumulate context partials.

                The broadcast runs on the otherwise-idle GpSimd engine except
                on the kernel's final chunk, where the PE is idle and the
                ones-matmul + cast path has lower latency.
                """
                ebc = ebcp.tile([128, S_CHUNK], BF16, tag="ebc")
                if tail:
                    bc_ps = ps_misc.tile([128, S_CHUNK], F32, tag="misc")
                    nc.tensor.matmul(
                        bc_ps[:], lhsT=ones_bf[:], rhs=ex[:], start=True, stop=True
                    )
                    nc.vector.tensor_copy(ebc[:], bc_ps[:])
                else:
                    nc.gpsimd.partition_broadcast(ebc[:], ex[:])
                for k in range(kt):
                    if USE_TTR:
                        trash = trashp.tile([128, S_CHUNK], BF16, tag="trash")
                        nc.vector.tensor_tensor_reduce(
                            trash[:],
                            xtb_bc[:, k, :],
                            ebc[:],
                            scale=1.0,
                            scalar=0.0,
                            op0=ALU.mult,
                            op1=ALU.add,
                            accum_out=ctx4_b[:, k * n_sc + c : k * n_sc + c + 1],
                        )
                    else:
                        scr = trashp.tile([128, S_CHUNK], BF16, tag="trash")
                        nc.vector.tensor_mul(scr[:], xtb_bc[:, k, :], ebc[:])
                        nc.vector.reduce_sum(
                            ctx4_b[:, k * n_sc + c : k * n_sc + c + 1],
                            scr[:],
                            axis=mybir.AxisListType.X,
                        )

            def emit_invd(denom_b):
                """softmax denominator -> broadcast 1/d [128, 1]."""
                dsum = smallp.tile([1, 1], F32, tag="dsum")
                nc.vector.reduce_sum(dsum[:], denom_b[:], axis=mybir.AxisListType.X)
                invd = smallp.tile([1, 1], F32, tag="invd")
                nc.vector.reciprocal(invd[:], dsum[:])
                iv_ps = ps_misc.tile([128, S_CHUNK], F32, tag="misc")
                nc.tensor.matmul(
                    iv_ps[:, 0:1], lhsT=ones_f32[:], rhs=invd[:], start=True, stop=True
                )
                invd_bc = smallp.tile([128, 1], F32, tag="invdbc")
                nc.scalar.copy(invd_bc[:], iv_ps[:, 0:1])
                return invd_bc

            def emit_batch_final(b, ctx4_b, invd_bc):
                """Partial reduction, normalize, store."""
                ctxu = ctxp.tile([128, jt], F32, tag="ctxu")
                for k in range(kt):
                    nc.vector.reduce_sum(
                        ctxu[:, k : k + 1],
                        ctx4_b[:, k * n_sc : (k + 1) * n_sc],
                        axis=mybir.AxisListType.X,
                    )
                ctx_b = ctxp.tile([128, jt], F32, tag="ctx")
                nc.vector.tensor_scalar_mul(ctx_b[:], ctxu[:], invd_bc[:])
                nc.sync.dma_start(ctx_d[b], ctx_b[:])

            pending = []  # deferred (context-chunk | invd | batch-final)
            for b in range(b_per_core):
                xt8_tiles = []
                xtb_tiles = []
                for c in range(n_sc):
                    xt8_bc = xt8p.tile([128, kt, S_CHUNK], FP8, tag="xt8")
                    if b == 0 and c == 0:
                        # split the gate-opening chunk so the first matmul
                        # group starts on the early half
                        half = kt // 2
                        nc.sync.dma_start(xt8_bc[:, :half, :], xt8_d[b, c][:, :half, :])
                        nc.sync.dma_start(xt8_bc[:, half:, :], xt8_d[b, c][:, half:, :])
                    else:
                        nc.sync.dma_start(xt8_bc[:], xt8_d[b, c])
                    xt8_tiles.append(xt8_bc)
                    xtb_bc = xtbp.tile([128, kt, S_CHUNK], BF16, tag="xtb")
                    nc.scalar.dma_start(xtb_bc[:], xtb_d[b, c])
                    xtb_tiles.append(xtb_bc)

                denom_b = smallp.tile([1, n_sc], F32, tag="denom")
                ctx4_b = ctxp.tile([128, kt * n_sc], F32, tag="ctx4")
                for c in range(n_sc):
                    ls_ps = ps_ls.tile([1, S_CHUNK], F32, tag="ls")
                    score_pairs = []
                    for j in range(jt):
                        jp, jh = divmod(j, 2)
                        if jh == 0:
                            sc = scorep.tile([128, 2, S_CHUNK], FP8, tag="score")
                            score_pairs.append(sc)
                        mm_ps = ps_main.tile([128, S_CHUNK], F32, tag="main")
                        if MAIN_DR:
                            for kp in range(kt // 2):
                                nc.tensor.matmul(
                                    mm_ps[:],
                                    lhsT=we_sb[:, 2 * kp : 2 * kp + 2, j * 128 : (j + 1) * 128],
                                    rhs=xt8_tiles[c][:, 2 * kp : 2 * kp + 2, :],
                                    start=(kp == 0),
                                    stop=(kp == kt // 2 - 1),
                                    perf_mode=DR,
                                )
                        else:
                            for k in range(kt):
                                nc.tensor.matmul(
                                    mm_ps[:],
                                    lhsT=we_sb[:, k, j * 128 : (j + 1) * 128],
                                    rhs=xt8_tiles[c][:, k, :],
                                    start=(k == 0),
                                    stop=(k == kt - 1),
                                )
                        nc.scalar.activation(
                            score_pairs[jp][:, jh, :], mm_ps[:], AF.Tanh,
                            scale=1.0 / W_SCALE,
                            bias=bias_sb[:, j * b_per_core + b : j * b_per_core + b + 1],
                        )
                        if j == 0:
                            # deferred work from the previous chunk/batch is
                            # emitted right after the first matmul group, so
                            # its PE ops slot in early and the DVE context
                            # work overlaps this chunk's remaining groups
                            for fn in pending:
                                fn()
                            pending = []
                    if LS_DR:
                        for jp in range(jt // 2):
                            nc.tensor.matmul(
                                ls_ps[:],
                                lhsT=ws_sb[:, 2 * jp : 2 * jp + 2, 0:1],
                                rhs=score_pairs[jp][:],
                                start=(jp == 0),
                                stop=(jp == jt // 2 - 1),
                                perf_mode=DR,
                            )
                    else:
                        for j in range(jt):
                            nc.tensor.matmul(
                                ls_ps[:],
                                lhsT=ws_sb[:, j, 0:1],
                                rhs=score_pairs[j // 2][:, j % 2, :],
                                start=(j == 0),
                                stop=(j == jt - 1),
                            )
                    ex = smallp.tile([1, S_CHUNK], BF16, tag="exp")
                    nc.scalar.activation(
                        ex[:], ls_ps[:], AF.Exp, scale=1.0 / W_SCALE,
                        accum_out=denom_b[:, c : c + 1]
                    )

                    last_b = b == b_per_core - 1
                    ctx_fn = (
                        lambda xtb_bc=xtb_tiles[c], ex=ex, ctx4_b=ctx4_b, c=c,
                        tl=(last_b and c == n_sc - 1):
                        emit_context_chunk(xtb_bc, ex, ctx4_b, c, tail=tl)
                    )
                    if c < n_sc - 1:
                        pending.append(ctx_fn)
                    elif last_b:
                        # tail of the whole kernel: get 1/d going on the
                        # still-empty DVE queue, then the final context chunk
                        invd_bc = emit_invd(denom_b)
                        ctx_fn()
                        emit_batch_final(b, ctx4_b, invd_bc)
                    else:
                        def batch_tail(ctx_fn=ctx_fn, b=b, ctx4_b=ctx4_b,
                                       denom_b=denom_b):
                            invd_bc = emit_invd(denom_b)
                            ctx_fn()
                            emit_batch_final(b, ctx4_b, invd_bc)
                        pending.append(batch_tail)

    nc.compile()
    return nc


_CACHED = {}


def _get_program(key):
    if key not in _CACHED:
        _CACHED[key] = build_program(*key)
    return _CACHED[key]


def make_in_maps(encoder_out, decoder_hidden_state, We, be, Wd, bd, Ws, bs,
                 b_per_core=B_PER_CORE, s=S, h=H, n_cores=N_CORES):
    kt = h // 128
    jt = h // 128
    n_sc = s // S_CHUNK
    bf = ml_dtypes.bfloat16
    f8 = ml_dtypes.float8_e4m3

    we_a = np.ascontiguousarray(
        (We * W_SCALE).reshape(kt, 128, h).transpose(1, 0, 2)
    ).astype(f8)
    ws_a = np.zeros((128, jt, 16), dtype=np.float32)
    ws_a[:, :, 0] = (Ws[:, 0] * W_SCALE).reshape(jt, 128).T
    ws_a = ws_a.astype(f8)

    dec = decoder_hidden_state[0]  # [32, h]
    bias_all = (be + bd)[None, :] + dec @ Wd  # [32, h] fp32
    in_maps = []
    for i in range(n_cores):
        b0 = i * b_per_core
        xb = encoder_out[b0 : b0 + b_per_core]  # [b, s, h]
        # [b, c, s', k, p] -> [b, c, p, k, s']
        xt5 = np.ascontiguousarray(
            xb.reshape(b_per_core, n_sc, S_CHUNK, kt, 128).transpose(0, 1, 4, 3, 2)
        )
        bias_a = np.ascontiguousarray(
            bias_all[b0 : b0 + b_per_core].reshape(b_per_core, jt, 128).transpose(2, 1, 0)
        ).reshape(128, jt * b_per_core).astype(np.float32)
        in_maps.append({
            "xt8": xt5.astype(f8),
            "xtb": xt5.astype(bf),
            "we": we_a,
            "ws": ws_a,
            "bias": bias_a,
        })
    return in_maps


def kernel(encoder_out, decoder_hidden_state, We, be, Wd, bd, Ws, bs):
    encoder_out = np.asarray(encoder_out, dtype=np.float32)
    decoder_hidden_state = np.asarray(decoder_hidden_state, dtype=np.float32)
    We = np.asarray(We, dtype=np.float32)
    be = np.asarray(be, dtype=np.float32)
    Wd = np.asarray(Wd, dtype=np.float32)
    bd = np.asarray(bd, dtype=np.float32)
    Ws = np.asarray(Ws, dtype=np.float32)
    bs = np.asarray(bs, dtype=np.float32)

    nc = _get_program((B_PER_CORE, S, H))
    in_maps = make_in_maps(
        encoder_out, decoder_hidden_state, We, be, Wd, bd, Ws, bs
    )
    kwargs = {}
    if PROFILE["trace"]:
        kwargs = {"trace": True, "tmpdir": PROFILE["tmpdir"]}
    res = run_bass_kernel_spmd(nc, in_maps, list(range(N_CORES)), **kwargs)
    PROFILE["last_result"] = res

    out = np.empty((N_CORES * B_PER_CORE, H), dtype=np.float32)
    for i in range(N_CORES):
        ctx = res.results[i]["ctx"]  # [b, 128, jt]
        out[i * B_PER_CORE : (i + 1) * B_PER_CORE] = (
            ctx.transpose(0, 2, 1).reshape(B_PER_CORE, H)
        )
    return out


# revision 11
# speedup vs baseline: 1.6563x; 1.6433x over previous
"""Bahdanau attention fused kernel for Trainium2, 8-core data-parallel.

Reference computation (per batch b of 32, H=1024, S=2048):
    enc_score = encoder_out @ We + be                    [B, S, H]
    dec_score = dec @ Wd + bd                            [B, 1, H]
    score     = tanh(enc_score + dec_score)              [B, S, H]
    ls        = score @ Ws + bs                          [B, S, 1]
    w         = softmax(ls, axis=S)
    out       = sum_s w[b,s] * encoder_out[b,s,:]        [B, H]

Sharding: batch 32 -> 4 per core across 8 cores; weights replicated.
The tiny dec-score GEMM is folded into the host-side bias preparation:
bias[b] = be + bd + dec[b] @ Wd. bs is dropped (softmax shift-invariant).
No max-subtraction in softmax: |ls| <= 16.

fp8 version: the big X@We GEMM and the score@Ws projection run in
fp8e4m3 with perf_mode=DoubleRow (2 fp8 weights per PE cell -> one
matmul contracts 256 rows).  The context accumulation keeps a separate
bf16 copy of X and fuses multiply+reduce into single-pass
tensor_tensor_reduce ops on VectorE.

Per-core device layout (h-partitioned, prepared host-side):
    xt8  [4, 4, 128, 8, 512] fp8  xt8[b,c,p,k,s'] = X[b, c*512+s', k*128+p]
    xtb  [4, 4, 128, 8, 512] bf16 same values in bf16 (context path)
    we   [128, 8, 1024]      fp8  we[p,k,n]       = We[k*128+p, n]
    ws   [128, 8, 16]        fp8  ws[p,j,0]       = Ws[j*128+p, 0] (rest 0)
    bias [128, 32]           f32  bias[p, j*4+b]  = (be+bd+dec[b]@Wd)[j*128+p]
    out: ctx [4, 128, 8]     f32  ctx[b,p,j]      = out[b, j*128+p]

Device schedule per (b, c) chunk:
  - 8 j-tiles x 4 DoubleRow matmuls (k-pairs) -> PSUM [128, 512]
  - ScalarE evacuates with fused tanh(psum + bias[b,j]) -> fp8 score,
    written into [128, 2, 512] j-pair tiles
  - ls.T = 4 DoubleRow matmuls (ws j-pairs x score pairs) -> PSUM [1,512]
  - ScalarE exp (bf16) with fused accum_out denominator (fp32)
  - context: exp weights broadcast to 128 partitions on GpSimd, then per
    k-tile one fused tensor_tensor_reduce (VectorE) accumulates
    ctx partials; deferred one chunk so nothing blocks the PE stream.
"""

import numpy as np
import ml_dtypes

import concourse.tile as tile
from concourse import bacc, mybir
from concourse.bass_utils import run_bass_kernel_spmd

FP8 = mybir.dt.float8e4
BF16 = mybir.dt.bfloat16
F32 = mybir.dt.float32
AF = mybir.ActivationFunctionType
ALU = mybir.AluOpType
DR = mybir.MatmulPerfMode.DoubleRow

N_CORES = 8
H = 1024
S = 2048
B_PER_CORE = 4
S_CHUNK = 512

# We/Ws are uniform(-1/32, 1/32) — below e4m3's min normal 2^-6 they
# quantize to subnormals (3.5x the noise).  Scale them up by 64 before
# the fp8 cast and fold 1/64 into the ScalarE activation scale (free).
W_SCALE = 64.0

# Feature flags (HW bring-up bisection)
MAIN_DR = True   # fp8 DoubleRow for the X@We GEMM
LS_DR = True     # fp8 DoubleRow for the score@Ws projection
USE_TTR = False   # fused tensor_tensor_reduce for the context path

# test.py can flip this to get a profiled run; the grading path never does.
PROFILE = {"trace": False, "tmpdir": None}


def build_program(b_per_core=B_PER_CORE, s=S, h=H):
    kt = h // 128
    jt = h // 128
    n_sc = s // S_CHUNK
    nc = bacc.Bacc("TRN2", target_bir_lowering=False, debug=False)

    xt8_d = nc.dram_tensor(
        "xt8", [b_per_core, n_sc, 128, kt, S_CHUNK], FP8, kind="ExternalInput"
    ).ap()
    xtb_d = nc.dram_tensor(
        "xtb", [b_per_core, n_sc, 128, kt, S_CHUNK], BF16, kind="ExternalInput"
    ).ap()
    we_d = nc.dram_tensor("we", [128, kt, h], FP8, kind="ExternalInput").ap()
    ws_d = nc.dram_tensor("ws", [128, jt, 16], FP8, kind="ExternalInput").ap()
    bias_d = nc.dram_tensor(
        "bias", [128, jt * b_per_core], F32, kind="ExternalInput"
    ).ap()
    ctx_d = nc.dram_tensor("ctx", [b_per_core, 128, jt], F32, kind="ExternalOutput").ap()

    with tile.TileContext(nc) as tc:
        with (
            tc.tile_pool(name="consts", bufs=1) as consts,
            tc.tile_pool(name="xt8p", bufs=8) as xt8p,
            tc.tile_pool(name="xtbp", bufs=5) as xtbp,
            tc.tile_pool(name="scorep", bufs=10) as scorep,
            tc.tile_pool(name="smallp", bufs=2 * n_sc) as smallp,
            tc.tile_pool(name="ebcp", bufs=2 * n_sc) as ebcp,
            tc.tile_pool(name="trashp", bufs=2) as trashp,
            tc.tile_pool(name="ctxp", bufs=2) as ctxp,
            tc.tile_pool(name="ps_main", bufs=4, space="PSUM") as ps_main,
            tc.tile_pool(name="ps_ls", bufs=2, space="PSUM") as ps_ls,
            tc.tile_pool(name="ps_misc", bufs=2, space="PSUM") as ps_misc,
        ):
            # we goes FIRST on the sync ring, ahead of the xt stream: with
            # the scalar ring nearly empty, the sync ring gets all 16 SDMA
            # engines, so the first-matmul gate (we + xt[0,0]) clears at
            # full HBM bandwidth instead of splitting it with prefetch.
            we_sb = consts.tile([128, kt, h], FP8)
            nc.sync.dma_start(we_sb[:], we_d[:])
            ws_sb = consts.tile([128, jt, 16], FP8)
            nc.scalar.dma_start(ws_sb[:], ws_d[:])
            bias_sb = consts.tile([128, jt * b_per_core], F32)
            nc.scalar.dma_start(bias_sb[:], bias_d[:])
            ones_bf = consts.tile([1, 128], BF16)
            nc.vector.memset(ones_bf[:], 1.0)
            ones_f32 = consts.tile([1, 128], F32)
            nc.vector.memset(ones_f32[:], 1.0)

            def emit_context_chunk(xtb_bc, ex, ctx4_b, c, tail=False):
                """Broadcast chunk weights and accumulate context partials.

                The broadcast runs on the otherwise-idle GpSimd engine except
                on the kernel's final chunk, where the PE is idle and the
                ones-matmul + cast path has lower latency.
                """
                ebc = ebcp.tile([128, S_CHUNK], BF16, tag="ebc")
                if tail:
                    bc_ps = ps_misc.tile([128, S_CHUNK], F32, tag="misc")
                    nc.tensor.matmul(
                        bc_ps[:], lhsT=ones_bf[:], rhs=ex[:], start=True, stop=True
                    )
                    nc.vector.tensor_copy(ebc[:], bc_ps[:])
                else:
                    nc.gpsimd.partition_broadcast(ebc[:], ex[:])
                for k in range(kt):
                    if USE_TTR:
                        trash = trashp.tile([128, S_CHUNK], BF16, tag="trash")
                        nc.vector.tensor_tensor_reduce(
                            trash[:],
                            xtb_bc[:, k, :],
                            ebc[:],
                            scale=1.0,
                            scalar=0.0,
                            op0=ALU.mult,
                            op1=ALU.add,
                            accum_out=ctx4_b[:, k * n_sc + c : k * n_sc + c + 1],
                        )
                    else:
                        scr = trashp.tile([128, S_CHUNK], BF16, tag="trash")
                        nc.vector.tensor_mul(scr[:], xtb_bc[:, k, :], ebc[:])
                        nc.vector.reduce_sum(
                            ctx4_b[:, k * n_sc + c : k * n_sc + c + 1],
                            scr[:],
                            axis=mybir.AxisListType.X,
                        )

            def emit_invd(denom_b):
                """softmax denominator -> broadcast 1/d [128, 1]."""
                dsum = smallp.tile([1, 1], F32, tag="dsum")
                nc.vector.reduce_sum(dsum[:], denom_b[:], axis=mybir.AxisListType.X)
                invd = smallp.tile([1, 1], F32, tag="invd")
                nc.vector.reciprocal(invd[:], dsum[:])
                iv_ps = ps_misc.tile([128, S_CHUNK], F32, tag="misc")
                nc.tensor.matmul(
                    iv_ps[:, 0:1], lhsT=ones_f32[:], rhs=invd[:], start=True, stop=True
                )
                invd_bc = smallp.tile([128, 1], F32, tag="invdbc")
                nc.scalar.copy(invd_bc[:], iv_ps[:, 0:1])
                return invd_bc

            def emit_batch_final(b, ctx4_b, invd_bc):
                """Partial reduction, normalize, store."""
                ctxu = ctxp.tile([128, jt], F32, tag="ctxu")
                for k in range(kt):
                    nc.vector.reduce_sum(
                        ctxu[:, k : k + 1],
                        ctx4_b[:, k * n_sc : (k + 1) * n_sc],
                        axis=mybir.AxisListType.X,
                    )
                ctx_b = ctxp.tile([128, jt], F32, tag="ctx")
                nc.vector.tensor_scalar_mul(ctx_b[:], ctxu[:], invd_bc[:])
                nc.sync.dma_start(ctx_d[b], ctx_b[:])

            pending = []  # deferred (context-chunk | invd | batch-final)
            for b in range(b_per_core):
                xt8_tiles = []
                xtb_tiles = []
                for c in range(n_sc):
                    xt8_bc = xt8p.tile([128, kt, S_CHUNK], FP8, tag="xt8")
                    if b == 0 and c == 0:
                        # split the gate-opening chunk so the first matmul
                        # group starts on the early half
                        half = kt // 2
                        nc.sync.dma_start(xt8_bc[:, :half, :], xt8_d[b, c][:, :half, :])
                        nc.sync.dma_start(xt8_bc[:, half:, :], xt8_d[b, c][:, half:, :])
                    else:
                        nc.sync.dma_start(xt8_bc[:], xt8_d[b, c])
                    xt8_tiles.append(xt8_bc)
                    xtb_bc = xtbp.tile([128, kt, S_CHUNK], BF16, tag="xtb")
                    nc.scalar.dma_start(xtb_bc[:], xtb_d[b, c])
                    xtb_tiles.append(xtb_bc)

                denom_b = smallp.tile([1, n_sc], F32, tag="denom")
                ctx4_b = ctxp.tile([128, kt * n_sc], F32, tag="ctx4")
                for c in range(n_sc):
                    ls_ps = ps_ls.tile([1, S_CHUNK], F32, tag="ls")
                    score_pairs = []
                    for j in range(jt):
                        jp, jh = divmod(j, 2)
                        if jh == 0:
                            sc = scorep.tile([128, 2, S_CHUNK], FP8, tag="score")
                            score_pairs.append(sc)
                        mm_ps = ps_main.tile([128, S_CHUNK], F32, tag="main")
                        if MAIN_DR:
                            for kp in range(kt // 2):
                                nc.tensor.matmul(
                                    mm_ps[:],
                                    lhsT=we_sb[:, 2 * kp : 2 * kp + 2, j * 128 : (j + 1) * 128],
                                    rhs=xt8_tiles[c][:, 2 * kp : 2 * kp + 2, :],
                                    start=(kp == 0),
                                    stop=(kp == kt // 2 - 1),
                                    perf_mode=DR,
                                )
                        else:
                            for k in range(kt):
                                nc.tensor.matmul(
                                    mm_ps[:],
                                    lhsT=we_sb[:, k, j * 128 : (j + 1) * 128],
                                    rhs=xt8_tiles[c][:, k, :],
                                    start=(k == 0),
                                    stop=(k == kt - 1),
                                )
                        nc.scalar.activation(
                            score_pairs[jp][:, jh, :], mm_ps[:], AF.Tanh,
                            scale=1.0 / W_SCALE,
                            bias=bias_sb[:, j * b_per_core + b : j * b_per_core + b + 1],
                        )
                        if j == 0:
                            # deferred work from the previous chunk/batch is
                            # emitted right after the first matmul group, so
                            # its PE ops slot in early and the DVE context
                            # work overlaps this chunk's remaining groups
                            for fn in pending:
                                fn()
                            pending = []
                    if LS_DR:
                        for jp in range(jt // 2):
                            nc.tensor.matmul(
                                ls_ps[:],
                                lhsT=ws_sb[:, 2 * jp : 2 * jp + 2, 0:1],
                                rhs=score_pairs[jp][:],
                                start=(jp == 0),
                                stop=(jp == jt // 2 - 1),
                                perf_mode=DR,
                            )
                    else:
                        for j in range(jt):
                            nc.tensor.matmul(
                                ls_ps[:],
                                lhsT=ws_sb[:, j, 0:1],
                                rhs=score_pairs[j // 2][:, j % 2, :],
                                start=(j == 0),
                                stop=(j == jt - 1),
                            )
                    ex = smallp.tile([1, S_CHUNK], BF16, tag="exp")
                    nc.scalar.activation(
                        ex[:], ls_ps[:], AF.Exp, scale=1.0 / W_SCALE,
                        accum_out=denom_b[:, c : c + 1]
                    )

                    last_b = b == b_per_core - 1
                    ctx_fn = (
                        lambda xtb_bc=xtb_tiles[c], ex=ex, ctx4_b=ctx4_b, c=c,
                        tl=(last_b and c == n_sc - 1):
                        emit_context_chunk(xtb_bc, ex, ctx4_b, c, tail=tl)
                    )
                    if c < n_sc - 1:
                        pending.append(ctx_fn)
                    elif last_b:
                        # tail of the whole kernel: get 1/d going on the
                        # still-empty DVE queue, then the final context chunk
                        invd_bc = emit_invd(denom_b)
                        ctx_fn()
                        emit_batch_final(b, ctx4_b, invd_bc)
                    else:
                        def batch_tail(ctx_fn=ctx_fn, b=b, ctx4_b=ctx4_b,
                                       denom_b=denom_b):
                            invd_bc = emit_invd(denom_b)
                            ctx_fn()
                            emit_batch_final(b, ctx4_b, invd_bc)
                        pending.append(batch_tail)

    nc.compile()
    return nc


_CACHED = {}


def _get_program(key):
    if key not in _CACHED:
        _CACHED[key] = build_program(*key)
    return _CACHED[key]


def make_in_maps(encoder_out, decoder_hidden_state, We, be, Wd, bd, Ws, bs,
                 b_per_core=B_PER_CORE, s=S, h=H, n_cores=N_CORES):
    kt = h // 128
    jt = h // 128
    n_sc = s // S_CHUNK
    bf = ml_dtypes.bfloat16
    f8 = ml_dtypes.float8_e4m3

    we_a = np.ascontiguousarray(
        (We * W_SCALE).reshape(kt, 128, h).transpose(1, 0, 2)
    ).astype(f8)
    ws_a = np.zeros((128, jt, 16), dtype=np.float32)
    ws_a[:, :, 0] = (Ws[:, 0] * W_SCALE).reshape(jt, 128).T
    ws_a = ws_a.astype(f8)

    dec = decoder_hidden_state[0]  # [32, h]
    bias_all = (be + bd)[None, :] + dec @ Wd  # [32, h] fp32
    in_maps = []
    for i in range(n_cores):
        b0 = i * b_per_core
        xb = encoder_out[b0 : b0 + b_per_core]  # [b, s, h]
        # [b, c, s', k, p] -> [b, c, p, k, s']
        xt5 = np.ascontiguousarray(
            xb.reshape(b_per_core, n_sc, S_CHUNK, kt, 128).transpose(0, 1, 4, 3, 2)
        )
        bias_a = np.ascontiguousarray(
            bias_all[b0 : b0 + b_per_core].reshape(b_per_core, jt, 128).transpose(2, 1, 0)
        ).reshape(128, jt * b_per_core).astype(np.float32)
        in_maps.append({
            "xt8": xt5.astype(f8),
            "xtb": xt5.astype(bf),
            "we": we_a,
            "ws": ws_a,
            "bias": bias_a,
        })
    return in_maps


def kernel(encoder_out, decoder_hidden_state, We, be, Wd, bd, Ws, bs):
    encoder_out = np.asarray(encoder_out, dtype=np.float32)
    decoder_hidden_state = np.asarray(decoder_hidden_state, dtype=np.float32)
    We = np.asarray(We, dtype=np.float32)
    be = np.asarray(be, dtype=np.float32)
    Wd = np.asarray(Wd, dtype=np.float32)
    bd = np.asarray(bd, dtype=np.float32)
    Ws = np.asarray(Ws, dtype=np.float32)
    bs = np.asarray(bs, dtype=np.float32)

    nc = _get_program((B_PER_CORE, S, H))
    in_maps = make_in_maps(
        encoder_out, decoder_hidden_state, We, be, Wd, bd, Ws, bs
    )
    kwargs = {}
    if PROFILE["trace"]:
        kwargs = {"trace": True, "tmpdir": PROFILE["tmpdir"]}
    res = run_bass_kernel_spmd(nc, in_maps, list(range(N_CORES)), **kwargs)
    PROFILE["last_result"] = res

    out = np.empty((N_CORES * B_PER_CORE, H), dtype=np.float32)
    for i in range(N_CORES):
        ctx = res.results[i]["ctx"]  # [b, 128, jt]
        out[i * B_PER_CORE : (i + 1) * B_PER_CORE] = (
            ctx.transpose(0, 2, 1).reshape(B_PER_CORE, H)
        )
    return out


# revision 14
# speedup vs baseline: 1.7617x; 1.0637x over previous
"""Bahdanau attention fused kernel for Trainium2, 8-core data-parallel.

Reference computation (per batch b of 32, H=1024, S=2048):
    enc_score = encoder_out @ We + be                    [B, S, H]
    dec_score = dec @ Wd + bd                            [B, 1, H]
    score     = tanh(enc_score + dec_score)              [B, S, H]
    ls        = score @ Ws + bs                          [B, S, 1]
    w         = softmax(ls, axis=S)
    out       = sum_s w[b,s] * encoder_out[b,s,:]        [B, H]

Sharding: batch 32 -> 4 per core across 8 cores; weights replicated.
The tiny dec-score GEMM is folded into the host-side bias preparation:
bias[b] = be + bd + dec[b] @ Wd. bs is dropped (softmax shift-invariant).
No max-subtraction in softmax: |ls| <= 16.

fp8 version: the big X@We GEMM and the score@Ws projection run in
fp8e4m3 with perf_mode=DoubleRow (2 fp8 weights per PE cell -> one
matmul contracts 256 rows).  The context accumulation keeps a separate
bf16 copy of X and fuses multiply+reduce into single-pass
tensor_tensor_reduce ops on VectorE.

Per-core device layout (h-partitioned, prepared host-side):
    xt8  [4, 4, 128, 8, 512] fp8  xt8[b,c,p,k,s'] = X[b, c*512+s', k*128+p]
    xtb  [4, 4, 128, 8, 512] bf16 same values in bf16 (context path)
    we   [128, 8, 1024]      fp8  we[p,k,n]       = We[k*128+p, n]
    ws   [128, 8, 16]        fp8  ws[p,j,0]       = Ws[j*128+p, 0] (rest 0)
    bias [128, 32]           f32  bias[p, j*4+b]  = (be+bd+dec[b]@Wd)[j*128+p]
    out: ctx [4, 128, 8]     f32  ctx[b,p,j]      = out[b, j*128+p]

Device schedule per (b, c) chunk:
  - 8 j-tiles x 4 DoubleRow matmuls (k-pairs) -> PSUM [128, 512]
  - ScalarE evacuates with fused tanh(psum + bias[b,j]) -> fp8 score,
    written into [128, 2, 512] j-pair tiles
  - ls.T = 4 DoubleRow matmuls (ws j-pairs x score pairs) -> PSUM [1,512]
  - ScalarE exp (bf16) with fused accum_out denominator (fp32)
  - context: exp weights broadcast to 128 partitions on GpSimd, then per
    k-tile one fused tensor_tensor_reduce (VectorE) accumulates
    ctx partials; deferred one chunk so nothing blocks the PE stream.
"""

import numpy as np
import ml_dtypes

import concourse.tile as tile
from concourse import bacc, mybir
from concourse.bass_utils import run_bass_kernel_spmd

FP8 = mybir.dt.float8e4
BF16 = mybir.dt.bfloat16
F32 = mybir.dt.float32
AF = mybir.ActivationFunctionType
ALU = mybir.AluOpType
DR = mybir.MatmulPerfMode.DoubleRow

N_CORES = 8
H = 1024
S = 2048
B_PER_CORE = 4
S_CHUNK = 512

# We/Ws are uniform(-1/32, 1/32) — below e4m3's min normal 2^-6 they
# quantize to subnormals (3.5x the noise).  Scale them up by 64 before
# the fp8 cast and fold 1/64 into the ScalarE activation scale (free).
W_SCALE = 64.0

# Feature flags (HW bring-up bisection)
MAIN_DR = True   # fp8 DoubleRow for the X@We GEMM
LS_DR = True     # fp8 DoubleRow for the score@Ws projection
USE_AMR = True   # fused affine_mul_reduce for the context path

# test.py can flip this to get a profiled run; the grading path never does.
PROFILE = {"trace": False, "tmpdir": None}


def build_program(b_per_core=B_PER_CORE, s=S, h=H):
    kt = h // 128
    jt = h // 128
    n_sc = s // S_CHUNK
    nc = bacc.Bacc("TRN2", target_bir_lowering=False, debug=False)

    xt8_d = nc.dram_tensor(
        "xt8", [b_per_core, n_sc, 128, kt, S_CHUNK], FP8, kind="ExternalInput"
    ).ap()
    xtb_d = nc.dram_tensor(
        "xtb", [b_per_core, n_sc, 128, kt, S_CHUNK], BF16, kind="ExternalInput"
    ).ap()
    we_d = nc.dram_tensor("we", [128, kt, h], FP8, kind="ExternalInput").ap()
    ws_d = nc.dram_tensor("ws", [128, jt, 16], FP8, kind="ExternalInput").ap()
    bias_d = nc.dram_tensor(
        "bias", [128, jt * b_per_core], F32, kind="ExternalInput"
    ).ap()
    ctx_d = nc.dram_tensor("ctx", [b_per_core, 128, jt], F32, kind="ExternalOutput").ap()

    with tile.TileContext(nc) as tc:
        with (
            tc.tile_pool(name="consts", bufs=1) as consts,
            tc.tile_pool(name="xt8p", bufs=8) as xt8p,
            tc.tile_pool(name="xtbp", bufs=5) as xtbp,
            tc.tile_pool(name="scorep", bufs=10) as scorep,
            tc.tile_pool(name="smallp", bufs=2 * n_sc) as smallp,
            tc.tile_pool(name="ebcp", bufs=2 * n_sc) as ebcp,
            tc.tile_pool(name="trashp", bufs=2) as trashp,
            tc.tile_pool(name="ctxp", bufs=2) as ctxp,
            tc.tile_pool(name="ps_main", bufs=4, space="PSUM") as ps_main,
            tc.tile_pool(name="ps_ls", bufs=2, space="PSUM") as ps_ls,
            tc.tile_pool(name="ps_misc", bufs=2, space="PSUM") as ps_misc,
        ):
            # we goes FIRST on the sync ring, ahead of the xt stream: with
            # the scalar ring nearly empty, the sync ring gets all 16 SDMA
            # engines, so the first-matmul gate (we + xt[0,0]) clears at
            # full HBM bandwidth instead of splitting it with prefetch.
            we_sb = consts.tile([128, kt, h], FP8)
            nc.sync.dma_start(we_sb[:], we_d[:])
            ws_sb = consts.tile([128, jt, 16], FP8)
            nc.scalar.dma_start(ws_sb[:], ws_d[:])
            bias_sb = consts.tile([128, jt * b_per_core], F32)
            nc.scalar.dma_start(bias_sb[:], bias_d[:])
            ones_bf = consts.tile([1, 128], BF16)
            nc.vector.memset(ones_bf[:], 1.0)
            ones_f32 = consts.tile([1, 128], F32)
            nc.vector.memset(ones_f32[:], 1.0)

            def emit_context_chunk(xtb_bc, ex, ctx4_b, c, tail=False):
                """Broadcast chunk weights and accumulate context partials.

                The broadcast runs on the otherwise-idle GpSimd engine except
                on the kernel's final chunk, where the PE is idle and the
                ones-matmul + cast path has lower latency.
                """
                ebc = ebcp.tile([128, S_CHUNK], BF16, tag="ebc")
                if tail:
                    bc_ps = ps_misc.tile([128, S_CHUNK], F32, tag="misc")
                    nc.tensor.matmul(
                        bc_ps[:], lhsT=ones_bf[:], rhs=ex[:], start=True, stop=True
                    )
                    nc.vector.tensor_copy(ebc[:], bc_ps[:])
                else:
                    nc.gpsimd.partition_broadcast(ebc[:], ex[:])
                for k in range(kt):
                    if USE_AMR:
                        # fused (xtb * ebc) multiply + free-axis reduce in one
                        # DVE pass (custom-ucode op; the ISA-level
                        # TENSOR_TENSOR_REDUCE doesn't execute on this runtime)
                        trash = trashp.tile([128, S_CHUNK], BF16, tag="trash")
                        nc.vector.affine_mul_reduce(
                            trash[:],
                            ctx4_b[:, k * n_sc + c : k * n_sc + c + 1],
                            xtb_bc[:, k, :],
                            ebc[:],
                            scale=1.0,
                            bias=0.0,
                        )
                    else:
                        scr = trashp.tile([128, S_CHUNK], BF16, tag="trash")
                        nc.vector.tensor_mul(scr[:], xtb_bc[:, k, :], ebc[:])
                        nc.vector.reduce_sum(
                            ctx4_b[:, k * n_sc + c : k * n_sc + c + 1],
                            scr[:],
                            axis=mybir.AxisListType.X,
                        )

            def emit_invd(denom_b):
                """softmax denominator -> broadcast 1/d [128, 1].

                The partition spread runs on GpSimd: a PE ones-matmul here
                would sit in the PE FIFO waiting on the exp chain and stall
                the next batch's main matmul stream (~2us per batch + a HAM
                re-throttle).
                """
                dsum = smallp.tile([1, 1], F32, tag="dsum")
                nc.vector.reduce_sum(dsum[:], denom_b[:], axis=mybir.AxisListType.X)
                invd = smallp.tile([1, 1], F32, tag="invd")
                nc.vector.reciprocal(invd[:], dsum[:])
                invd_bc = smallp.tile([128, 1], F32, tag="invdbc")
                nc.gpsimd.partition_broadcast(invd_bc[:], invd[:])
                return invd_bc

            def emit_batch_final(b, ctx4_b, invd_bc):
                """Partial reduction, normalize, store."""
                ctxu = ctxp.tile([128, jt], F32, tag="ctxu")
                for k in range(kt):
                    nc.vector.reduce_sum(
                        ctxu[:, k : k + 1],
                        ctx4_b[:, k * n_sc : (k + 1) * n_sc],
                        axis=mybir.AxisListType.X,
                    )
                ctx_b = ctxp.tile([128, jt], F32, tag="ctx")
                nc.vector.tensor_scalar_mul(ctx_b[:], ctxu[:], invd_bc[:])
                nc.sync.dma_start(ctx_d[b], ctx_b[:])

            pending = []  # deferred (context-chunk | invd | batch-final)
            for b in range(b_per_core):
                xt8_tiles = []
                xtb_tiles = []
                for c in range(n_sc):
                    xt8_bc = xt8p.tile([128, kt, S_CHUNK], FP8, tag="xt8")
                    if b == 0 and c == 0:
                        # split the gate-opening chunk so the first matmul
                        # group starts on the early half
                        half = kt // 2
                        nc.sync.dma_start(xt8_bc[:, :half, :], xt8_d[b, c][:, :half, :])
                        nc.sync.dma_start(xt8_bc[:, half:, :], xt8_d[b, c][:, half:, :])
                    else:
                        nc.sync.dma_start(xt8_bc[:], xt8_d[b, c])
                    xt8_tiles.append(xt8_bc)
                    xtb_bc = xtbp.tile([128, kt, S_CHUNK], BF16, tag="xtb")
                    nc.scalar.dma_start(xtb_bc[:], xtb_d[b, c])
                    xtb_tiles.append(xtb_bc)

                denom_b = smallp.tile([1, n_sc], F32, tag="denom")
                ctx4_b = ctxp.tile([128, kt * n_sc], F32, tag="ctx4")
                for c in range(n_sc):
                    ls_ps = ps_ls.tile([1, S_CHUNK], F32, tag="ls")
                    score_pairs = []
                    for j in range(jt):
                        jp, jh = divmod(j, 2)
                        if jh == 0:
                            sc = scorep.tile([128, 2, S_CHUNK], FP8, tag="score")
                            score_pairs.append(sc)
                        mm_ps = ps_main.tile([128, S_CHUNK], F32, tag="main")
                        if MAIN_DR:
                            for kp in range(kt // 2):
                                nc.tensor.matmul(
                                    mm_ps[:],
                                    lhsT=we_sb[:, 2 * kp : 2 * kp + 2, j * 128 : (j + 1) * 128],
                                    rhs=xt8_tiles[c][:, 2 * kp : 2 * kp + 2, :],
                                    start=(kp == 0),
                                    stop=(kp == kt // 2 - 1),
                                    perf_mode=DR,
                                )
                        else:
                            for k in range(kt):
                                nc.tensor.matmul(
                                    mm_ps[:],
                                    lhsT=we_sb[:, k, j * 128 : (j + 1) * 128],
                                    rhs=xt8_tiles[c][:, k, :],
                                    start=(k == 0),
                                    stop=(k == kt - 1),
                                )
                        nc.scalar.activation(
                            score_pairs[jp][:, jh, :], mm_ps[:], AF.Tanh,
                            scale=1.0 / W_SCALE,
                            bias=bias_sb[:, j * b_per_core + b : j * b_per_core + b + 1],
                        )
                        if j == 0:
                            # deferred work from the previous chunk/batch is
                            # emitted right after the first matmul group, so
                            # its PE ops slot in early and the DVE context
                            # work overlaps this chunk's remaining groups
                            for fn in pending:
                                fn()
                            pending = []
                    if LS_DR:
                        for jp in range(jt // 2):
                            nc.tensor.matmul(
                                ls_ps[:],
                                lhsT=ws_sb[:, 2 * jp : 2 * jp + 2, 0:1],
                                rhs=score_pairs[jp][:],
                                start=(jp == 0),
                                stop=(jp == jt // 2 - 1),
                                perf_mode=DR,
                            )
                    else:
                        for j in range(jt):
                            nc.tensor.matmul(
                                ls_ps[:],
                                lhsT=ws_sb[:, j, 0:1],
                                rhs=score_pairs[j // 2][:, j % 2, :],
                                start=(j == 0),
                                stop=(j == jt - 1),
                            )
                    ex = smallp.tile([1, S_CHUNK], BF16, tag="exp")
                    nc.scalar.activation(
                        ex[:], ls_ps[:], AF.Exp, scale=1.0 / W_SCALE,
                        accum_out=denom_b[:, c : c + 1]
                    )

                    last_b = b == b_per_core - 1
                    ctx_fn = (
                        lambda xtb_bc=xtb_tiles[c], ex=ex, ctx4_b=ctx4_b, c=c,
                        tl=(last_b and c == n_sc - 1):
                        emit_context_chunk(xtb_bc, ex, ctx4_b, c, tail=tl)
                    )
                    if c < n_sc - 1:
                        pending.append(ctx_fn)
                    elif last_b:
                        # tail of the whole kernel: get 1/d going on the
                        # still-empty DVE queue, then the final context chunk
                        invd_bc = emit_invd(denom_b)
                        ctx_fn()
                        emit_batch_final(b, ctx4_b, invd_bc)
                    else:
                        def batch_tail(ctx_fn=ctx_fn, b=b, ctx4_b=ctx4_b,
                                       denom_b=denom_b):
                            invd_bc = emit_invd(denom_b)
                            ctx_fn()
                            emit_batch_final(b, ctx4_b, invd_bc)
                        pending.append(batch_tail)

    nc.compile()
    return nc


_CACHED = {}


def _get_program(key):
    if key not in _CACHED:
        _CACHED[key] = build_program(*key)
    return _CACHED[key]


def make_in_maps(encoder_out, decoder_hidden_state, We, be, Wd, bd, Ws, bs,
                 b_per_core=B_PER_CORE, s=S, h=H, n_cores=N_CORES):
    kt = h // 128
    jt = h // 128
    n_sc = s // S_CHUNK
    bf = ml_dtypes.bfloat16
    f8 = ml_dtypes.float8_e4m3

    we_a = np.ascontiguousarray(
        (We * W_SCALE).reshape(kt, 128, h).transpose(1, 0, 2)
    ).astype(f8)
    ws_a = np.zeros((128, jt, 16), dtype=np.float32)
    ws_a[:, :, 0] = (Ws[:, 0] * W_SCALE).reshape(jt, 128).T
    ws_a = ws_a.astype(f8)

    dec = decoder_hidden_state[0]  # [32, h]
    bias_all = (be + bd)[None, :] + dec @ Wd  # [32, h] fp32
    in_maps = []
    for i in range(n_cores):
        b0 = i * b_per_core
        xb = encoder_out[b0 : b0 + b_per_core]  # [b, s, h]
        # [b, c, s', k, p] -> [b, c, p, k, s']
        xt5 = np.ascontiguousarray(
            xb.reshape(b_per_core, n_sc, S_CHUNK, kt, 128).transpose(0, 1, 4, 3, 2)
        )
        bias_a = np.ascontiguousarray(
            bias_all[b0 : b0 + b_per_core].reshape(b_per_core, jt, 128).transpose(2, 1, 0)
        ).reshape(128, jt * b_per_core).astype(np.float32)
        in_maps.append({
            "xt8": xt5.astype(f8),
            "xtb": xt5.astype(bf),
            "we": we_a,
            "ws": ws_a,
            "bias": bias_a,
        })
    return in_maps


def kernel(encoder_out, decoder_hidden_state, We, be, Wd, bd, Ws, bs):
    encoder_out = np.asarray(encoder_out, dtype=np.float32)
    decoder_hidden_state = np.asarray(decoder_hidden_state, dtype=np.float32)
    We = np.asarray(We, dtype=np.float32)
    be = np.asarray(be, dtype=np.float32)
    Wd = np.asarray(Wd, dtype=np.float32)
    bd = np.asarray(bd, dtype=np.float32)
    Ws = np.asarray(Ws, dtype=np.float32)
    bs = np.asarray(bs, dtype=np.float32)

    nc = _get_program((B_PER_CORE, S, H))
    in_maps = make_in_maps(
        encoder_out, decoder_hidden_state, We, be, Wd, bd, Ws, bs
    )
    kwargs = {}
    if PROFILE["trace"]:
        kwargs = {"trace": True, "tmpdir": PROFILE["tmpdir"]}
    res = run_bass_kernel_spmd(nc, in_maps, list(range(N_CORES)), **kwargs)
    PROFILE["last_result"] = res

    out = np.empty((N_CORES * B_PER_CORE, H), dtype=np.float32)
    for i in range(N_CORES):
        ctx = res.results[i]["ctx"]  # [b, 128, jt]
        out[i * B_PER_CORE : (i + 1) * B_PER_CORE] = (
            ctx.transpose(0, 2, 1).reshape(B_PER_CORE, H)
        )
    return out


# revision 19
# speedup vs baseline: 1.8529x; 1.0518x over previous
"""Bahdanau attention fused kernel for Trainium2, 8-core data-parallel.

Reference computation (per batch b of 32, H=1024, S=2048):
    enc_score = encoder_out @ We + be                    [B, S, H]
    dec_score = dec @ Wd + bd                            [B, 1, H]
    score     = tanh(enc_score + dec_score)              [B, S, H]
    ls        = score @ Ws + bs                          [B, S, 1]
    w         = softmax(ls, axis=S)
    out       = sum_s w[b,s] * encoder_out[b,s,:]        [B, H]

Sharding: batch 32 -> 4 per core across 8 cores; weights replicated.
The tiny dec-score GEMM is folded into the host-side bias preparation:
bias[b] = be + bd + dec[b] @ Wd. bs is dropped (softmax shift-invariant).
No max-subtraction in softmax: |ls| <= 16.

fp8 version: the big X@We GEMM and the score@Ws projection run in
fp8e4m3 with perf_mode=DoubleRow (2 fp8 weights per PE cell -> one
matmul contracts 256 rows).  The context accumulation keeps a separate
bf16 copy of X and fuses multiply+reduce into single-pass
tensor_tensor_reduce ops on VectorE.

Per-core device layout (h-partitioned, prepared host-side):
    xt8  [4, 4, 128, 8, 512] fp8  xt8[b,c,p,k,s'] = X[b, c*512+s', k*128+p]
    xtb  [4, 4, 128, 8, 512] bf16 same values in bf16 (context path)
    we   [128, 8, 1024]      fp8  we[p,k,n]       = We[k*128+p, n]
    ws   [128, 8, 16]        fp8  ws[p,j,0]       = Ws[j*128+p, 0] (rest 0)
    bias [128, 32]           f32  bias[p, j*4+b]  = (be+bd+dec[b]@Wd)[j*128+p]
    out: ctx [4, 128, 8]     f32  ctx[b,p,j]      = out[b, j*128+p]

Device schedule per (b, c) chunk:
  - 8 j-tiles x 4 DoubleRow matmuls (k-pairs) -> PSUM [128, 512]
  - ScalarE evacuates with fused tanh(psum + bias[b,j]) -> fp8 score,
    written into [128, 2, 512] j-pair tiles
  - ls.T = 4 DoubleRow matmuls (ws j-pairs x score pairs) -> PSUM [1,512]
  - ScalarE exp (bf16) with fused accum_out denominator (fp32)
  - context: exp weights broadcast to 128 partitions on GpSimd, then per
    k-tile one fused tensor_tensor_reduce (VectorE) accumulates
    ctx partials; deferred one chunk so nothing blocks the PE stream.
"""

import numpy as np
import ml_dtypes

import concourse.tile as tile
from concourse import bacc, mybir
from concourse.bass_utils import run_bass_kernel_spmd

FP8 = mybir.dt.float8e4
BF16 = mybir.dt.bfloat16
F32 = mybir.dt.float32
AF = mybir.ActivationFunctionType
ALU = mybir.AluOpType
DR = mybir.MatmulPerfMode.DoubleRow

N_CORES = 8
H = 1024
S = 2048
B_PER_CORE = 4
S_CHUNK = 512

# We/Ws are uniform(-1/32, 1/32) — below e4m3's min normal 2^-6 they
# quantize to subnormals (3.5x the noise).  Scale them up by 64 before
# the fp8 cast and fold 1/64 into the ScalarE activation scale (free).
W_SCALE = 64.0

# Feature flags (HW bring-up bisection)
MAIN_DR = True   # fp8 DoubleRow for the X@We GEMM
LS_DR = True     # fp8 DoubleRow for the score@Ws projection
USE_AMR = True   # fused affine_mul_reduce for the context path

# test.py can flip this to get a profiled run; the grading path never does.
PROFILE = {"trace": False, "tmpdir": None}


def build_program(b_per_core=B_PER_CORE, s=S, h=H):
    kt = h // 128
    jt = h // 128
    n_sc = s // S_CHUNK
    nc = bacc.Bacc("TRN2", target_bir_lowering=False, debug=False)

    xt8_d = nc.dram_tensor(
        "xt8", [b_per_core, n_sc, 128, kt, S_CHUNK], FP8, kind="ExternalInput"
    ).ap()
    xtb_d = nc.dram_tensor(
        "xtb", [b_per_core, n_sc, 128, kt, S_CHUNK], BF16, kind="ExternalInput"
    ).ap()
    we_d = nc.dram_tensor("we", [128, kt, h], FP8, kind="ExternalInput").ap()
    ws_d = nc.dram_tensor("ws", [128, jt, 16], FP8, kind="ExternalInput").ap()
    bias_d = nc.dram_tensor(
        "bias", [128, jt * b_per_core], F32, kind="ExternalInput"
    ).ap()
    ctx_d = nc.dram_tensor("ctx", [b_per_core, 128, jt], F32, kind="ExternalOutput").ap()

    with tile.TileContext(nc) as tc:
        with (
            tc.tile_pool(name="consts", bufs=1) as consts,
            tc.tile_pool(name="xt8p", bufs=8) as xt8p,
            tc.tile_pool(name="xtbp", bufs=5) as xtbp,
            tc.tile_pool(name="scorep", bufs=10) as scorep,
            tc.tile_pool(name="smallp", bufs=2 * n_sc) as smallp,
            tc.tile_pool(name="ebcp", bufs=2 * n_sc) as ebcp,
            tc.tile_pool(name="trashp", bufs=2) as trashp,
            tc.tile_pool(name="ctxp", bufs=2) as ctxp,
            # 6 main banks let the PE run two j-groups ahead of the tanh
            # evacuations; ls gets the other 2 (the tail broadcast borrows
            # a buf from ls since it runs after the last ls group).
            tc.tile_pool(name="ps_main", bufs=6, space="PSUM") as ps_main,
            tc.tile_pool(name="ps_ls", bufs=1, space="PSUM") as ps_ls,
            tc.tile_pool(name="ps_tail", bufs=1, space="PSUM") as ps_tail,
        ):
            # we goes FIRST on the sync ring, split by k-pair and
            # interleaved with the first xt8 chunk's k-pairs below, so the
            # first matmul gates on 384 KB, not the full 1.5 MB.
            we_sb = consts.tile([128, kt, h], FP8)
            for kp in range(kt // 2):
                nc.sync.dma_start(
                    we_sb[:, 2 * kp : 2 * kp + 2, :], we_d[:, 2 * kp : 2 * kp + 2, :]
                )
            ws_sb = consts.tile([128, jt, 16], FP8)
            nc.scalar.dma_start(ws_sb[:], ws_d[:])
            bias_sb = consts.tile([128, jt * b_per_core], F32)
            nc.scalar.dma_start(bias_sb[:], bias_d[:])
            ones_bf = consts.tile([1, 128], BF16)
            nc.vector.memset(ones_bf[:], 1.0)
            ones_f32 = consts.tile([1, 128], F32)
            nc.vector.memset(ones_f32[:], 1.0)

            def emit_context_chunk(xtb_bc, ex, ctx4_b, c, tail=False):
                """Broadcast chunk weights and accumulate context partials.

                The broadcast runs on the otherwise-idle GpSimd engine except
                on the kernel's final chunk, where the PE is idle and the
                ones-matmul + cast path has lower latency.
                """
                if tail:
                    # PE is idle at the tail: ones-matmul broadcast into PSUM
                    # and let the AMRs read it from there (saves the copy)
                    ebc = ps_tail.tile([128, S_CHUNK], F32, tag="tailbc")
                    nc.tensor.matmul(
                        ebc[:], lhsT=ones_bf[:], rhs=ex[:], start=True, stop=True
                    )
                else:
                    ebc = ebcp.tile([128, S_CHUNK], BF16, tag="ebc")
                    nc.gpsimd.partition_broadcast(ebc[:], ex[:])
                for k in range(kt):
                    if USE_AMR:
                        # fused (xtb * ebc) multiply + free-axis reduce in one
                        # DVE pass (custom-ucode op; the ISA-level
                        # TENSOR_TENSOR_REDUCE doesn't execute on this runtime)
                        trash = trashp.tile([128, S_CHUNK], BF16, tag="trash")
                        nc.vector.affine_mul_reduce(
                            trash[:],
                            ctx4_b[:, k * n_sc + c : k * n_sc + c + 1],
                            xtb_bc[:, k, :],
                            ebc[:],
                            scale=1.0,
                            bias=0.0,
                        )
                    else:
                        scr = trashp.tile([128, S_CHUNK], BF16, tag="trash")
                        nc.vector.tensor_mul(scr[:], xtb_bc[:, k, :], ebc[:])
                        nc.vector.reduce_sum(
                            ctx4_b[:, k * n_sc + c : k * n_sc + c + 1],
                            scr[:],
                            axis=mybir.AxisListType.X,
                        )

            def emit_invd(denom_b):
                """softmax denominator -> broadcast 1/d [128, 1].

                The partition spread runs on GpSimd: a PE ones-matmul here
                would sit in the PE FIFO waiting on the exp chain and stall
                the next batch's main matmul stream (~2us per batch + a HAM
                re-throttle).
                """
                dsum = smallp.tile([1, 1], F32, tag="dsum")
                nc.vector.reduce_sum(dsum[:], denom_b[:], axis=mybir.AxisListType.X)
                invd = smallp.tile([1, 1], F32, tag="invd")
                nc.vector.reciprocal(invd[:], dsum[:])
                invd_bc = smallp.tile([128, 1], F32, tag="invdbc")
                nc.gpsimd.partition_broadcast(invd_bc[:], invd[:])
                return invd_bc

            def emit_batch_final(b, ctx4_b, invd_bc):
                """Partial reduction, normalize, store."""
                ctxu = ctxp.tile([128, jt], F32, tag="ctxu")
                for k in range(kt):
                    nc.vector.reduce_sum(
                        ctxu[:, k : k + 1],
                        ctx4_b[:, k * n_sc : (k + 1) * n_sc],
                        axis=mybir.AxisListType.X,
                    )
                ctx_b = ctxp.tile([128, jt], F32, tag="ctx")
                nc.vector.tensor_scalar_mul(ctx_b[:], ctxu[:], invd_bc[:])
                nc.sync.dma_start(ctx_d[b], ctx_b[:])

            pending = []  # deferred (context-chunk | invd | batch-final)
            for b in range(b_per_core):
                xt8_tiles = []
                xtb_tiles = []
                for c in range(n_sc):
                    xt8_bc = xt8p.tile([128, kt, S_CHUNK], FP8, tag="xt8")
                    if b == 0 and c == 0:
                        # split the gate-opening chunk by k-pair so the
                        # first matmul group starts on the first slice
                        for kp in range(kt // 2):
                            nc.sync.dma_start(
                                xt8_bc[:, 2 * kp : 2 * kp + 2, :],
                                xt8_d[b, c][:, 2 * kp : 2 * kp + 2, :],
                            )
                    else:
                        nc.sync.dma_start(xt8_bc[:], xt8_d[b, c])
                    xt8_tiles.append(xt8_bc)
                    # xtb rides the sync queue too: a dma_start costs ~0.6us
                    # on its issuing engine queue, and on ScalarE that issue
                    # cost delayed tanh evacuations enough to stall the PE
                    # at every batch boundary.
                    xtb_bc = xtbp.tile([128, kt, S_CHUNK], BF16, tag="xtb")
                    nc.sync.dma_start(xtb_bc[:], xtb_d[b, c])
                    xtb_tiles.append(xtb_bc)

                denom_b = smallp.tile([1, n_sc], F32, tag="denom")
                ctx4_b = ctxp.tile([128, kt * n_sc], F32, tag="ctx4")
                for c in range(n_sc):
                    ls_ps = ps_ls.tile([1, S_CHUNK], F32, tag="ls")
                    score_pairs = []
                    for j in range(jt):
                        jp, jh = divmod(j, 2)
                        if jh == 0:
                            sc = scorep.tile([128, 2, S_CHUNK], FP8, tag="score")
                            score_pairs.append(sc)
                        mm_ps = ps_main.tile([128, S_CHUNK], F32, tag="main")
                        if MAIN_DR:
                            for kp in range(kt // 2):
                                nc.tensor.matmul(
                                    mm_ps[:],
                                    lhsT=we_sb[:, 2 * kp : 2 * kp + 2, j * 128 : (j + 1) * 128],
                                    rhs=xt8_tiles[c][:, 2 * kp : 2 * kp + 2, :],
                                    start=(kp == 0),
                                    stop=(kp == kt // 2 - 1),
                                    perf_mode=DR,
                                )
                        else:
                            for k in range(kt):
                                nc.tensor.matmul(
                                    mm_ps[:],
                                    lhsT=we_sb[:, k, j * 128 : (j + 1) * 128],
                                    rhs=xt8_tiles[c][:, k, :],
                                    start=(k == 0),
                                    stop=(k == kt - 1),
                                )
                        nc.scalar.activation(
                            score_pairs[jp][:, jh, :], mm_ps[:], AF.Tanh,
                            scale=1.0 / W_SCALE,
                            bias=bias_sb[:, j * b_per_core + b : j * b_per_core + b + 1],
                        )
                        if j == 0:
                            # deferred work from the previous chunk/batch is
                            # emitted right after the first matmul group, so
                            # its PE ops slot in early and the DVE context
                            # work overlaps this chunk's remaining groups
                            for fn in pending:
                                fn()
                            pending = []
                    if LS_DR:
                        for jp in range(jt // 2):
                            nc.tensor.matmul(
                                ls_ps[:],
                                lhsT=ws_sb[:, 2 * jp : 2 * jp + 2, 0:1],
                                rhs=score_pairs[jp][:],
                                start=(jp == 0),
                                stop=(jp == jt // 2 - 1),
                                perf_mode=DR,
                            )
                    else:
                        for j in range(jt):
                            nc.tensor.matmul(
                                ls_ps[:],
                                lhsT=ws_sb[:, j, 0:1],
                                rhs=score_pairs[j // 2][:, j % 2, :],
                                start=(j == 0),
                                stop=(j == jt - 1),
                            )
                    ex = smallp.tile([1, S_CHUNK], BF16, tag="exp")
                    nc.scalar.activation(
                        ex[:], ls_ps[:], AF.Exp, scale=1.0 / W_SCALE,
                        accum_out=denom_b[:, c : c + 1]
                    )

                    last_b = b == b_per_core - 1
                    ctx_fn = (
                        lambda xtb_bc=xtb_tiles[c], ex=ex, ctx4_b=ctx4_b, c=c,
                        tl=(last_b and c == n_sc - 1):
                        emit_context_chunk(xtb_bc, ex, ctx4_b, c, tail=tl)
                    )
                    if c < n_sc - 1:
                        pending.append(ctx_fn)
                    elif last_b:
                        # tail of the whole kernel: get 1/d going on the
                        # still-empty DVE queue, then the final context chunk
                        invd_bc = emit_invd(denom_b)
                        ctx_fn()
                        emit_batch_final(b, ctx4_b, invd_bc)
                    else:
                        def batch_tail(ctx_fn=ctx_fn, b=b, ctx4_b=ctx4_b,
                                       denom_b=denom_b):
                            invd_bc = emit_invd(denom_b)
                            ctx_fn()
                            emit_batch_final(b, ctx4_b, invd_bc)
                        pending.append(batch_tail)

    nc.compile()
    return nc


_CACHED = {}


def _get_program(key):
    if key not in _CACHED:
        _CACHED[key] = build_program(*key)
    return _CACHED[key]


def make_in_maps(encoder_out, decoder_hidden_state, We, be, Wd, bd, Ws, bs,
                 b_per_core=B_PER_CORE, s=S, h=H, n_cores=N_CORES):
    kt = h // 128
    jt = h // 128
    n_sc = s // S_CHUNK
    bf = ml_dtypes.bfloat16
    f8 = ml_dtypes.float8_e4m3

    we_a = np.ascontiguousarray(
        (We * W_SCALE).reshape(kt, 128, h).transpose(1, 0, 2)
    ).astype(f8)
    ws_a = np.zeros((128, jt, 16), dtype=np.float32)
    ws_a[:, :, 0] = (Ws[:, 0] * W_SCALE).reshape(jt, 128).T
    ws_a = ws_a.astype(f8)

    dec = decoder_hidden_state[0]  # [32, h]
    bias_all = (be + bd)[None, :] + dec @ Wd  # [32, h] fp32
    in_maps = []
    for i in range(n_cores):
        b0 = i * b_per_core
        xb = encoder_out[b0 : b0 + b_per_core]  # [b, s, h]
        # [b, c, s', k, p] -> [b, c, p, k, s']
        xt5 = np.ascontiguousarray(
            xb.reshape(b_per_core, n_sc, S_CHUNK, kt, 128).transpose(0, 1, 4, 3, 2)
        )
        bias_a = np.ascontiguousarray(
            bias_all[b0 : b0 + b_per_core].reshape(b_per_core, jt, 128).transpose(2, 1, 0)
        ).reshape(128, jt * b_per_core).astype(np.float32)
        in_maps.append({
            "xt8": xt5.astype(f8),
            "xtb": xt5.astype(bf),
            "we": we_a,
            "ws": ws_a,
            "bias": bias_a,
        })
    return in_maps


def kernel(encoder_out, decoder_hidden_state, We, be, Wd, bd, Ws, bs):
    encoder_out = np.asarray(encoder_out, dtype=np.float32)
    decoder_hidden_state = np.asarray(decoder_hidden_state, dtype=np.float32)
    We = np.asarray(We, dtype=np.float32)
    be = np.asarray(be, dtype=np.float32)
    Wd = np.asarray(Wd, dtype=np.float32)
    bd = np.asarray(bd, dtype=np.float32)
    Ws = np.asarray(Ws, dtype=np.float32)
    bs = np.asarray(bs, dtype=np.float32)

    nc = _get_program((B_PER_CORE, S, H))
    in_maps = make_in_maps(
        encoder_out, decoder_hidden_state, We, be, Wd, bd, Ws, bs
    )
    kwargs = {}
    if PROFILE["trace"]:
        kwargs = {"trace": True, "tmpdir": PROFILE["tmpdir"]}
    res = run_bass_kernel_spmd(nc, in_maps, list(range(N_CORES)), **kwargs)
    PROFILE["last_result"] = res

    out = np.empty((N_CORES * B_PER_CORE, H), dtype=np.float32)
    for i in range(N_CORES):
        ctx = res.results[i]["ctx"]  # [b, 128, jt]
        out[i * B_PER_CORE : (i + 1) * B_PER_CORE] = (
            ctx.transpose(0, 2, 1).reshape(B_PER_CORE, H)
        )
    return out
